# revision 24
# baseline (speedup 1.0000x reference)
"""Trainium2 Bass kernel for a 2-layer heterogeneous GNN (HHGN).

Graph: authors (100k x 128), teams (25k x 128).
Relations: aa (author->author, 1.6M edges), at (author->team, 400k),
ta (team->author, 400k). GraphConv norm='both' with per-relation degree
normalization, HeteroGraphConv aggregate='sum', two layers with shared
weights.

Strategy (8 NeuronCores, SPMD):
 - Host: relabel nodes (per ntype) into 128-node blocks dealt by degree so
   every block has a near-equal edge count; shard blocks across the 8
   cores; partition every relation's edge list by destination-block owner;
   fold the symmetric degree normalization into a per-edge weight.
 - Device, per destination block: indirect-DMA gather of source rows
   (512B/row), segment-sum on the PE array via a one-hot selection matrix
   built on the fly by the vector engine ((iota == local_dst) * w), the
   aggregate stays transposed in PSUM so the per-relation 128x128 weight
   matmul lands output rows directly; relation outputs accumulate in PSUM.
 - Features for both ntypes live in one packed, replicated table so both
   layers use identical gather indices. Between layers an AllGather
   rebuilds the full table from the 8 per-core shards.
"""

import os
import numpy as np

import concourse.bass as bass
import concourse.bacc as bacc
import concourse.mybir as mybir
import concourse.tile as tile
from concourse.bass import IndirectOffsetOnAxis
from concourse.bass_utils import run_bass_kernel_spmd

P = 128
D = 128
C = 8

N_AUTHOR = 100000
N_TEAM = 25000

# blocks per core (authors / teams); capacities C*BA*P, C*BT*P
BA = (N_AUTHOR + C * P - 1) // (C * P)  # 98
BT = (N_TEAM + C * P - 1) // (C * P)    # 25
SA = BA * P      # 12544 author rows per core
ST = BT * P      # 3200 team rows per core
S = SA + ST      # 15744 packed rows per core
NTAB = C * S     # 125952 packed table rows

LAST_INFO: dict = {}


def _deal_blocks(weights: np.ndarray, n_blocks: int):
    """Assign each node to a block, snake-dealing by descending weight so
    block weight sums are near-equal. Returns (block_id, slot) per node."""
    n = weights.shape[0]
    order = np.argsort(-weights, kind="stable")
    i = np.arange(n)
    r = i // n_blocks
    j = i % n_blocks
    b = np.where(r % 2 == 0, j, n_blocks - 1 - j)
    block_id = np.empty(n, np.int64)
    slot_of = np.empty(n, np.int64)
    block_id[order] = b
    slot_of[order] = r
    assert slot_of.max() < P
    return block_id, slot_of


def _preprocess(h_author, h_team, aa_src, aa_dst, at_src, at_dst,
                ta_src, ta_dst):
    f32 = np.float32

    def norm(src, dst, n_src, n_dst):
        od = np.maximum(np.bincount(src, minlength=n_src), 1).astype(f32)
        idg = np.maximum(np.bincount(dst, minlength=n_dst), 1).astype(f32)
        return (1.0 / np.sqrt(od))[src] * (1.0 / np.sqrt(idg))[dst]

    w_aa = norm(aa_src, aa_dst, N_AUTHOR, N_AUTHOR)
    w_at = norm(at_src, at_dst, N_AUTHOR, N_TEAM)
    w_ta = norm(ta_src, ta_dst, N_TEAM, N_AUTHOR)

    # node -> (block, slot): balance author blocks on total landing edges
    a_weight = (np.bincount(aa_dst, minlength=N_AUTHOR)
                + np.bincount(ta_dst, minlength=N_AUTHOR)).astype(np.int64)
    t_weight = np.bincount(at_dst, minlength=N_TEAM).astype(np.int64)
    a_block, a_slot = _deal_blocks(a_weight, C * BA)
    t_block, t_slot = _deal_blocks(t_weight, C * BT)

    # packed-table row per node (core-major: [core][author rows][team rows])
    a_core = a_block // BA
    t_core = t_block // BT
    a_packed = a_core * S + (a_block % BA) * P + a_slot
    t_packed = t_core * S + SA + (t_block % BT) * P + t_slot

    h0 = np.zeros((NTAB, D), f32)
    h0[a_packed] = h_author
    h0[t_packed] = h_team

    def build(src_packed_ids, db, ds_, w_edge, n_blocks_total,
              blocks_per_core):
        order = np.lexsort((src_packed_ids, db))
        sb = db[order]
        counts = np.bincount(sb, minlength=n_blocks_total)
        starts = np.zeros(n_blocks_total + 1, np.int64)
        np.cumsum(counts, out=starts[1:])
        rank = np.arange(sb.shape[0]) - starts[sb]
        T = int((counts.max() + P - 1) // P)
        K = blocks_per_core * T
        gi = np.zeros((C, P, K), np.int32)
        lw = np.zeros((C, P, 2 * K), f32)
        core = sb // blocks_per_core
        blocal = sb % blocks_per_core
        t = rank // P
        p = rank % P
        col = blocal * T + t
        gi[core, p, col] = src_packed_ids[order]
        lw[core, p, col] = ds_[order].astype(f32)
        lw[core, p, K + col] = w_edge[order]
        return T, gi, lw

    T_AA, gi_aa, lw_aa = build(a_packed[aa_src], a_block[aa_dst],
                               a_slot[aa_dst], w_aa, C * BA, BA)
    T_TA, gi_ta, lw_ta = build(t_packed[ta_src], a_block[ta_dst],
                               a_slot[ta_dst], w_ta, C * BA, BA)
    T_AT, gi_at, lw_at = build(a_packed[at_src], t_block[at_dst],
                               t_slot[at_dst], w_at, C * BT, BT)

    a_row_global = a_core * SA + (a_block % BA) * P + a_slot
    t_row_global = t_core * ST + (t_block % BT) * P + t_slot

    return dict(h0=h0, T_AA=T_AA, T_TA=T_TA, T_AT=T_AT,
                gi_aa=gi_aa, lw_aa=lw_aa, gi_ta=gi_ta, lw_ta=lw_ta,
                gi_at=gi_at, lw_at=lw_at,
                a_row_global=a_row_global, t_row_global=t_row_global)


def _build_program(T_AA, T_TA, T_AT):
    dt = mybir.dt
    nc = bacc.Bacc()
    KAA, KTA, KAT = BA * T_AA, BA * T_TA, BT * T_AT

    h0 = nc.dram_tensor("h0", [NTAB, D], dt.float32, kind="ExternalInput")
    gi_aa = nc.dram_tensor("gi_aa", [P, KAA], dt.int32, kind="ExternalInput")
    lw_aa = nc.dram_tensor("lw_aa", [P, 2 * KAA], dt.float32,
                           kind="ExternalInput")
    gi_ta = nc.dram_tensor("gi_ta", [P, KTA], dt.int32, kind="ExternalInput")
    lw_ta = nc.dram_tensor("lw_ta", [P, 2 * KTA], dt.float32,
                           kind="ExternalInput")
    gi_at = nc.dram_tensor("gi_at", [P, KAT], dt.int32, kind="ExternalInput")
    lw_at = nc.dram_tensor("lw_at", [P, 2 * KAT], dt.float32,
                           kind="ExternalInput")
    # [0:3D) W_aa|W_ta|W_at, [3D:4D) bias_author, [4D:5D) bias_team,
    # [5D:6D) iota row 0..127 replicated on every partition
    meta = nc.dram_tensor("meta", [P, 6 * D], dt.float32,
                          kind="ExternalInput")
    out_a = nc.dram_tensor("out_a", [SA, D], dt.float32, kind="ExternalOutput")
    out_t = nc.dram_tensor("out_t", [ST, D], dt.float32, kind="ExternalOutput")

    eq = mybir.AluOpType.is_equal
    mul = mybir.AluOpType.mult
    add = mybir.AluOpType.add

    with tile.TileContext(nc) as tc:
        with (
            tc.tile_pool(name="dram", bufs=1, space="DRAM") as dram,
            tc.tile_pool(name="const", bufs=1) as const,
            tc.tile_pool(name="gbuf", bufs=3) as gpool,
            tc.tile_pool(name="onehot", bufs=3) as spool,
            tc.tile_pool(name="aggsb", bufs=4) as apool,
            tc.tile_pool(name="outsb", bufs=4) as opool,
            tc.tile_pool(name="psum_agg", bufs=4, space="PSUM") as ppool,
            tc.tile_pool(name="psum_out", bufs=2, space="PSUM") as pout,
        ):
            h1s = dram.tile([S, D], dt.float32)
            h1 = dram.tile([NTAB, D], dt.float32, addr_space="Shared")

            meta_sb = const.tile([P, 6 * D], dt.float32)
            nc.sync.dma_start(out=meta_sb[:], in_=meta[:])
            w_sb = meta_sb[:, 0:3 * D]
            bias_a = meta_sb[:, 3 * D:4 * D]
            bias_t = meta_sb[:, 4 * D:5 * D]
            iota_f = meta_sb[:, 5 * D:6 * D]

            def load_rel(gi, lw, K, name):
                g = const.tile([P, K], dt.int32, name=f"gi_{name}")
                l = const.tile([P, 2 * K], dt.float32, name=f"lw_{name}")
                nc.sync.dma_start(out=g[:], in_=gi[:])
                nc.sync.dma_start(out=l[:], in_=lw[:])
                return g, l

            rel_aa = load_rel(gi_aa, lw_aa, KAA, "aa")
            rel_ta = load_rel(gi_ta, lw_ta, KTA, "ta")
            rel_at = load_rel(gi_at, lw_at, KAT, "at")

            # replicated iota: 0..127 repeated TMAX times along the free dim
            TMAX = max(T_AA, T_TA, T_AT)
            iota_rep_i = const.tile([P, TMAX * P], dt.int32)
            nc.gpsimd.iota(iota_rep_i[:], pattern=[[0, TMAX], [1, P]],
                           base=0, channel_multiplier=0)
            iota_rep = const.tile([P, TMAX * P], dt.float32)
            nc.vector.tensor_copy(iota_rep[:], iota_rep_i[:])


            def half_layer(src_tab, n_blocks, rels, out_dram, bias_sb):
                for b in range(n_blocks):
                    aggs = []
                    for (T, K, g_sb, l_sb, w_off, tag) in rels:
                        gb = gpool.tile([P, T * P], dt.float32,
                                        tag=f"g_{tag}")
                        # HW consumes one index per partition per indirect
                        # DMA (free dim = contiguous bytes), so gather one
                        # 128-edge tile per call
                        for t in range(T):
                            nc.gpsimd.indirect_dma_start(
                                out=gb[:, t * P:(t + 1) * P],
                                out_offset=None, in_=src_tab[:, :],
                                in_offset=IndirectOffsetOnAxis(
                                    ap=g_sb[:, b * T + t:b * T + t + 1],
                                    axis=0))
                        ps = ppool.tile([P, P], dt.float32, tag="agg")
                        s_all = spool.tile([P, T * P], dt.float32,
                                           tag=f"s_{tag}")
                        s3 = s_all[:].rearrange("p (t d) -> p t d", d=P)
                        ld_bc = l_sb[:, b * T:(b + 1) * T].to_broadcast(
                            [P, T, P])
                        w_bc = l_sb[:, K + b * T:K + (b + 1) * T].to_broadcast(
                            [P, T, P])
                        i3 = iota_rep[:, 0:T * P].rearrange(
                            "p (t d) -> p t d", d=P)
                        nc.vector.tensor_tensor(out=s3, in0=ld_bc, in1=i3,
                                                op=eq)
                        nc.vector.tensor_tensor(out=s3, in0=s3, in1=w_bc,
                                                op=mul)
                        for t in range(T):
                            nc.tensor.matmul(ps[:],
                                             lhsT=gb[:, t * P:(t + 1) * P],
                                             rhs=s_all[:, t * P:(t + 1) * P],
                                             start=(t == 0),
                                             stop=(t == T - 1))
                        a_sb = apool.tile([P, P], dt.float32, tag="agg_sb")
                        nc.vector.tensor_copy(a_sb[:], ps[:])
                        aggs.append((a_sb, w_off))
                    ops_ = pout.tile([P, P], dt.float32, tag="out_ps")
                    for i, (a_sb, w_off) in enumerate(aggs):
                        nc.tensor.matmul(ops_[:], lhsT=a_sb[:],
                                         rhs=w_sb[:, w_off:w_off + D],
                                         start=(i == 0),
                                         stop=(i == len(aggs) - 1))
                    o_sb = opool.tile([P, P], dt.float32, tag="o_sb")
                    nc.vector.tensor_tensor(out=o_sb[:], in0=ops_[:],
                                            in1=bias_sb, op=add)
                    nc.sync.dma_start(out=out_dram[b * P:(b + 1) * P, :],
                                      in_=o_sb[:])

            arels = [(T_AA, KAA, *rel_aa, 0 * D, "aa"),
                     (T_TA, KTA, *rel_ta, 1 * D, "ta")]
            trels = [(T_AT, KAT, *rel_at, 2 * D, "at")]

            # layer 1 -> per-core shard of the packed table
            half_layer(h0, BA, arels, h1s[0:SA, :], bias_a)
            half_layer(h0, BT, trels, h1s[SA:S, :], bias_t)

            nc.gpsimd.collective_compute(
                "AllGather", mybir.AluOpType.bypass,
                replica_groups=[list(range(C))],
                ins=[h1s[:]], outs=[h1[:]])

            # layer 2 -> external outputs
            half_layer(h1, BA, arels, out_a[:, :], bias_a)
            half_layer(h1, BT, trels, out_t[:, :], bias_t)

    nc.finalize()
    return nc


def _make_meta(W_aa, W_ta, W_at, b_aa, b_ta, b_at):
    meta = np.zeros((P, 6 * D), np.float32)
    meta[:, 0:D] = W_aa
    meta[:, D:2 * D] = W_ta
    meta[:, 2 * D:3 * D] = W_at
    meta[:, 3 * D:4 * D] = (b_aa + b_ta)[None, :]
    meta[:, 4 * D:5 * D] = b_at[None, :]
    meta[:, 5 * D:6 * D] = np.arange(D, dtype=np.float32)[None, :]
    return meta


def kernel(h_author, h_team, aa_src, aa_dst, at_src, at_dst, ta_src, ta_dst,
           W_aa, b_aa, W_at, b_at, W_ta, b_ta):
    pre = _preprocess(
        np.asarray(h_author, np.float32), np.asarray(h_team, np.float32),
        np.asarray(aa_src, np.int64), np.asarray(aa_dst, np.int64),
        np.asarray(at_src, np.int64), np.asarray(at_dst, np.int64),
        np.asarray(ta_src, np.int64), np.asarray(ta_dst, np.int64))

    meta = _make_meta(np.asarray(W_aa, np.float32),
                      np.asarray(W_ta, np.float32),
                      np.asarray(W_at, np.float32),
                      np.asarray(b_aa, np.float32),
                      np.asarray(b_ta, np.float32),
                      np.asarray(b_at, np.float32))

    nc = _build_program(pre["T_AA"], pre["T_TA"], pre["T_AT"])

    in_maps = []
    for c in range(C):
        in_maps.append({
            "h0": pre["h0"],
            "gi_aa": pre["gi_aa"][c], "lw_aa": pre["lw_aa"][c],
            "gi_ta": pre["gi_ta"][c], "lw_ta": pre["lw_ta"][c],
            "gi_at": pre["gi_at"][c], "lw_at": pre["lw_at"][c],
            "meta": meta,
        })

    import time
    t0 = time.perf_counter()
    res = run_bass_kernel_spmd(nc, in_maps, core_ids=list(range(C)),
                               trace=bool(os.environ.get("KBENCH_TRACE")))
    LAST_INFO["wall_ns"] = int((time.perf_counter() - t0) * 1e9)
    LAST_INFO["exec_time_ns"] = res.exec_time_ns
    LAST_INFO["results"] = res

    out_a_all = np.concatenate([res.results[c]["out_a"] for c in range(C)],
                               axis=0)
    out_t_all = np.concatenate([res.results[c]["out_t"] for c in range(C)],
                               axis=0)
    h2_author = out_a_all[pre["a_row_global"]]
    h2_team = out_t_all[pre["t_row_global"]]
    return h2_author, h2_team
